# revision 113
# baseline (speedup 1.0000x reference)
"""Trainium2 Bass kernel for DynamicGNN (3-layer RGCN-style message passing).

Strategy: shard destination nodes (and their incoming edges) across the 8
NeuronCores. Each core owns N/8 nodes = 3*N/8 (node,relation) segments.
Messages are gathered per-edge from a replicated f16 PAIR-ROW node table
([N/2, 128] f16: two nodes share one 256B dma_gather row; idx = pair_row,
and tiles are split per window into even-src / odd-src classes so the
valid half is a compile-time AP slice). Segment reduction runs on the
TensorEngine: per 128-edge tile a f16 selection mask (iota==rel) is
loaded stationary and the f16 messages stream, so PSUM accumulates
[segment, feature]; mean denominators are a per-partition scalar multiply
on the PSUM drain, and one transpose matmul per 128-segment window lands
the result feature-major in S_T. The per-layer table rebuild AllGathers
compact f16 rows directly into the pair-row table in two halves (no
local expand pass). Per-relation transform/root/bias/relu/LayerNorm run
feature-major in chunks of 512 nodes.

Host->device traffic is kept minimal: per core a static "graph" blob
([16, X] int8: compact int16 gather indices, int8 per-slot window-local
segment ids, f16 per-segment inverse counts), a "wts" blob (f16 weights)
and a per-call "xin" blob (f16 x row-image); all replicated/bitcast/
converted on device. The compiled executable is cached per graph shape,
and graph/wts device buffers are cached across calls that pass identical
arrays (serving mode); output buffers are donated back each call.
Output is int8 with a per-row f16 scale packed in the same tensor
(dequantized on host)."""
import os
import sys

import numpy as np

DBG_NO_GATHER = os.environ.get("DBG_NO_GATHER") == "1"
DBG_NO_COLLECTIVE = os.environ.get("DBG_NO_COLLECTIVE") == "1"

sys.path.insert(0, "/opt/trn_rl_repo")

NCORES = 8
WINSEG = 128          # segments per psum window
BATCH_TILES = 8       # 128-edge tiles per dma_gather call (desc-ring limit ~1024)
LN_EPS = 1e-5
NUM_REL = 3

O = 64
L = 3
IN_DIM = 5

# const array column layout ([64, CC] f32)
C_F2CW = 0            # rows 0:5, cols 0:64
C_RGCNW = 64          # rows 0:64, 576 cols ((l*3+r)*64)
C_ROOTW = 640         # rows 0:64, 192 cols
C_BIAST = 832         # rows 0:64, 3 cols
C_F2CB = 835          # rows 0:64, 1 col
C_GAMMA = 836         # row 0, 64 cols
C_BETA = 900          # row 0, 64 cols
CC = 964


def _ceil(a, b):
    return (a + b - 1) // b


def _preprocess(x, edge_index, edge_type):
    """Host-side: shard edges by dst owner, sort by segment, build windows,
    per-phase slot streams (gather idx + per-slot rel/count metadata)."""
    N = x.shape[0]
    E = edge_index.shape[1]
    n_own = N // NCORES
    seg_per_core = n_own * NUM_REL
    nwin = _ceil(seg_per_core, WINSEG)

    src = edge_index[0].astype(np.int64)
    dst = edge_index[1].astype(np.int64)
    et = edge_type.astype(np.int64)

    assert n_own % 2 == 0, "pair-row gather table needs even n_own"
    # the per-layer AllGather is split in two so the first half overlaps the
    # tail of the previous stage; table layout = [all cores' first H1 nodes |
    # all cores' last H2 nodes], both halves even-sized for pair rows
    H1 = max(128, (n_own // 2 // 128) * 128)   # 128-aligned, even
    H2 = n_own - H1
    R1 = NCORES * (H1 // 2)        # pair rows in region 1

    def pair_row(s):
        c, p = s // n_own, s % n_own
        return np.where(p < H1, c * (H1 // 2) + (p >> 1),
                        R1 + c * (H2 // 2) + ((p - H1) >> 1))

    owner = dst // n_own
    cntE = np.zeros((NCORES, nwin), dtype=np.int64)
    cntO = np.zeros((NCORES, nwin), dtype=np.int64)
    per_core = []
    for c in range(NCORES):
        m = owner == c
        s_c = src[m]
        seg_c = (dst[m] - c * n_own) * NUM_REL + et[m]
        # sort by (segment, src): src-sorted runs compress better on the wire
        order = np.lexsort((s_c, seg_c))
        s_c = s_c[order]
        seg_c = seg_c[order]
        w_c = seg_c // WINSEG
        # parity classes: even-src tiles read the low half of the pair row,
        # odd-src tiles the high half (compile-time AP slice, no merge ops)
        isE = (s_c & 1) == 0
        cntE[c] = np.bincount(w_c[isE], minlength=nwin)
        cntO[c] = np.bincount(w_c[~isE], minlength=nwin)
        # per-(local segment) counts for mean denominators
        segcnt = np.bincount(seg_c, minlength=seg_per_core)
        per_core.append((s_c, seg_c, w_c, isE, segcnt))

    # compile-time tile structure: tiles per (parity, window) = max over cores
    tilesA = _ceil(np.maximum(cntE.max(axis=0), 0), 128)   # [nwin]
    tilesB = _ceil(np.maximum(cntO.max(axis=0), 0), 128)
    TA, TB = int(tilesA.sum()), int(tilesB.sum())
    T = TA + TB
    slotsA = TA * 128
    tbaseA = np.concatenate([[0], np.cumsum(tilesA)[:-1]])
    tbaseB = np.concatenate([[0], np.cumsum(tilesB)[:-1]])

    cores = []
    for c in range(NCORES):
        s_c, seg_c, w_c, isE, segcnt = per_core[c]
        idx_s = np.zeros(T * 128, dtype=np.int16)
        rel_s = np.full(T * 128, -1, dtype=np.int8)
        for (mask, tbase, soff) in ((isE, tbaseA, 0), (~isE, tbaseB, slotsA)):
            s_p = s_c[mask]
            seg_p = seg_c[mask]
            w_p = w_c[mask]
            gc = np.bincount(w_p, minlength=nwin)
            starts = np.concatenate([[0], np.cumsum(gc)[:-1]])
            pos = np.arange(len(w_p)) - starts[w_p]
            slot = soff + tbase[w_p] * 128 + pos
            idx_s[slot] = pair_row(s_p).astype(np.int16)
            rel_s[slot] = (seg_p - w_p * WINSEG).astype(np.int8)
        # per-segment inverse counts (mean denominators), f16 on the wire,
        # laid out [seg-in-window, window] for the device
        inv = np.ones(nwin * WINSEG, dtype=np.float16)
        inv[:seg_per_core] = (
            1.0 / np.maximum(segcnt, 1).astype(np.float64)).astype(np.float16)
        invT = np.ascontiguousarray(inv.reshape(nwin, WINSEG).T)  # [128,nwin]
        inv16 = (invT.view(np.int8).reshape(8, 16, 2 * nwin)
                 .transpose(1, 0, 2).reshape(16, 16 * nwin))

        # idx: [S] -> [16, S/16]; rel: [S] -> [128, S/128] -> regroup
        relp = rel_s.reshape(-1, 128).T
        rel16 = relp.reshape(8, 16, T).transpose(1, 0, 2).reshape(16, 8 * T)
        idx16 = np.ascontiguousarray(idx_s.reshape(-1, 16).T)
        blob = np.concatenate([idx16.view(np.int8), rel16, inv16], axis=1)
        cores.append(np.ascontiguousarray(blob))

    segp_pad = _ceil(nwin * WINSEG, 3 * 512) * (3 * 512)
    meta = dict(
        N=N, E=E, n_own=n_own, seg_per_core=seg_per_core, nwin=nwin,
        tilesA=tilesA, tilesB=tilesB, TA=TA, TB=TB, segp_pad=segp_pad,
        H1=H1, H2=H2,
    )
    return meta, cores, None


def _build_program(meta):
    import concourse.bacc as bacc
    import concourse.mybir as mybir
    import concourse.tile as tile
    from concourse.masks import make_identity

    dt = mybir.dt
    f32 = dt.float32
    f16 = dt.float16
    N = meta["N"]
    n_own = meta["n_own"]
    nwin = meta["nwin"]
    tilesA, tilesB = meta["tilesA"], meta["tilesB"]
    TA, TB = meta["TA"], meta["TB"]
    T = TA + TB
    segp_pad = meta["segp_pad"]
    H1, H2 = meta["H1"], meta["H2"]
    R1 = NCORES * (H1 // 2)

    nc = bacc.Bacc("TRN2", target_bir_lowering=False, debug=False,
                   enable_asserts=False, num_devices=NCORES)

    IOFF = 24 * T                          # inverse-count region byte offset
    IVR = 2 * nwin                         # inv bytes per 16-row group
    IVB = 8 * IVR                          # inv bytes per row
    XCH = _ceil(n_own, 128)                # 128-node chunks of x
    graph_d = nc.dram_tensor("graph", [16, IOFF + IVB], dt.int8,
                             kind="ExternalInput")
    wts_d = nc.dram_tensor("wts", [16, 8 * CC], dt.int8,
                           kind="ExternalInput")
    xin_d = nc.dram_tensor("xin", [16, 8 * XCH * IN_DIM * 2], dt.int8,
                           kind="ExternalInput")
    out_d = nc.dram_tensor("out", [n_own, O + 2], dt.int8,
                           kind="ExternalOutput")

    AluOp = mybir.AluOpType
    Act = mybir.ActivationFunctionType

    with tile.TileContext(nc) as tc:
        with (
            tc.tile_pool(name="persist", bufs=1) as pp,
            tc.tile_pool(name="msgpA", bufs=5) as msgpA,
            tc.tile_pool(name="msgpB", bufs=5) as msgpB,
            tc.tile_pool(name="selp", bufs=8) as selp,
            tc.tile_pool(name="denp", bufs=2) as denp,
            tc.tile_pool(name="rowp", bufs=6) as rowp,
            tc.tile_pool(name="lnp", bufs=8) as lnp,
            tc.tile_pool(name="strp", bufs=3) as strp,
            tc.tile_pool(name="psw", bufs=3, space="PSUM") as psw,
            tc.tile_pool(name="pspost", bufs=2, space="PSUM") as pspost,
            tc.tile_pool(name="pstr", bufs=2, space="PSUM") as pstr,
            tc.tile_pool(name="pstb", bufs=1, space="PSUM") as pstb,
            tc.tile_pool(name="dram", bufs=1, space="DRAM") as dr,
        ):
            def persist(name, shape, d=f32):
                return pp.tile(shape, d, tag=name, name=name)

            idx_sb = persist("idx_sb", [128, T * 8], dt.int16)
            rel8 = persist("rel8", [128, T], dt.int8)
            konst16 = persist("konst16", [O, CC], f16)
            ximg = persist("ximg", [128, XCH * IN_DIM], f16)
            ximg32 = persist("ximg32", [128, XCH * IN_DIM])
            magicT = persist("magicT", [128, O])
            relf = persist("relf", [128, T])
            inv16 = persist("inv16", [128, nwin], f16)
            invf = persist("invf", [128, nwin])
            iota = persist("iota", [128, 128])
            iota16 = persist("iota16", [128, 128], f16)
            ident = persist("ident", [128, 128])
            konst = persist("konst", [O, CC])
            gammaB = persist("gammaB", [128, O])
            betaB = persist("betaB", [128, O])
            onesrow = persist("onesrow", [1, 128])
            S_T = persist("S_T", [O, segp_pad])
            hT = persist("hT", [O, n_own])
            epscol = persist("epscol", [128, 1])

            idx_src = graph_d[:, 0:16 * T].bitcast(dt.int16)
            for g in range(8):
                nc.sync.dma_start(idx_sb[g * 16:(g + 1) * 16, :], idx_src)
                nc.sync.dma_start(
                    rel8[g * 16:(g + 1) * 16, :],
                    graph_d[:, 16 * T + g * T:16 * T + (g + 1) * T])
            for g in range(8):
                nc.sync.dma_start(
                    inv16[g * 16:(g + 1) * 16, :],
                    graph_d[:, IOFF + g * IVR:IOFF + (g + 1) * IVR]
                    .bitcast(f16))
            nc.scalar.activation(invf[:], inv16[:], Act.Copy)
            for g in range(4):
                nc.sync.dma_start(
                    konst16[g * 16:(g + 1) * 16, :],
                    wts_d[:, g * 2 * CC:(g + 1) * 2 * CC].bitcast(f16))
            nc.scalar.activation(konst[:], konst16[:], Act.Copy)
            XR = XCH * IN_DIM * 2
            for g in range(8):
                nc.sync.dma_start(
                    ximg[g * 16:(g + 1) * 16, :],
                    xin_d[:, g * XR:(g + 1) * XR].bitcast(f16))
            nc.scalar.activation(ximg32[:], ximg[:], Act.Copy)
            nc.vector.memset(magicT[:], 12582912.0)

            make_identity(nc, ident[:])
            nc.gpsimd.iota(iota[:], [[1, 128]], channel_multiplier=0,
                           allow_small_or_imprecise_dtypes=True)
            nc.scalar.activation(iota16[:], iota[:], Act.Copy)
            nc.vector.memset(epscol[:], LN_EPS)
            nc.vector.memset(onesrow[:], 1.0)
            nc.vector.memset(S_T[:], 0.0)

            # decode slot metadata: rel (f32)
            nc.vector.tensor_scalar(out=relf[:], in0=rel8[:],
                                    scalar1=0.0, scalar2=None, op0=AluOp.add)

            # broadcast gamma/beta rows across 128 partitions via ones-matmul
            for (col, dst) in ((C_GAMMA, gammaB), (C_BETA, betaB)):
                psg = pstr.tile([128, 128], f32, tag="ptr", name="psg")
                nc.tensor.matmul(psg[:, :O], onesrow[:],
                                 konst[0:1, col:col + O], start=True, stop=True)
                nc.scalar.activation(dst[:], psg[:, :O], Act.Copy)

            # DRAM internals: per-layer bounce -> pair-row AllGather target.
            # dma_gather rows must be 256B-aligned, so the gather table packs
            # TWO nodes per row ([N/2, 128] f16); idx = src>>1 and the valid
            # half is selected per edge on the vector engine (parity merge).
            bounceA = [dr.tile([H1, O], f16, tag=f"bounceA{l}",
                               name=f"bounceA{l}") for l in range(L)]
            bounceB = [dr.tile([H2, O], f16, tag=f"bounceB{l}",
                               name=f"bounceB{l}") for l in range(L)]
            ctable = [dr.tile([N // 2, 2 * O], f16, tag=f"ctable{l}",
                              name=f"ctable{l}") for l in range(L)]

            def chunks(total, step):
                return [(i, min(step, total - i)) for i in range(0, total, step)]

            # ---- layer 0 node features: h0T = f2cW.T @ xT (+bias) ----
            ximg3 = ximg32[:].rearrange("p (k d) -> p k d", d=IN_DIM)
            for ki, (o, n2) in enumerate(chunks(n_own, 128)):
                psX = pstr.tile([128, 128], f32, tag="ptr", name="psX")
                nc.tensor.matmul(psX[:IN_DIM, :n2], ximg3[:n2, ki, :],
                                 ident[:n2, :n2], start=True, stop=True)
                xTc = strp.tile([IN_DIM, 128], f32, tag="xTc", name="xTc")
                nc.scalar.activation(xTc[:, :n2], psX[:IN_DIM, :n2], Act.Copy)
                ps = pspost.tile([O, 512], f32, tag="pspost", name="ps")
                nc.tensor.matmul(ps[:, :n2], konst[0:IN_DIM, 0:O], xTc[:, :n2],
                                 start=True, stop=True)
                nc.scalar.activation(hT[:, o:o + n2], ps[:, :n2], Act.Identity,
                                     bias=konst[:, C_F2CB:C_F2CB + 1])

            def build_table(l):
                """transpose hT columns into f16 row chunks, DMA to the two
                bounce halves, AllGather each into the pair-row table."""
                for (o, n) in chunks(n_own, 128):
                    ps = pstb.tile([128, 128], f32, tag="ptb", name="ps")
                    nc.tensor.matmul(ps[:n, :O], hT[:, o:o + n], ident[:O, :O],
                                     start=True, stop=True)
                    rows = rowp.tile([128, O], f16, tag="rows16",
                                     name="rows")
                    nc.scalar.activation(rows[:n, :], ps[:n, :O], Act.Copy)
                    if o < H1:
                        nc.sync.dma_start(bounceA[l][o:o + n, :], rows[:n, :])
                    else:
                        nc.sync.dma_start(bounceB[l][o - H1:o - H1 + n, :],
                                          rows[:n, :])
                if DBG_NO_COLLECTIVE:
                    nc.sync.dma_start(ctable[l][0:H1 // 2, :], bounceA[l][:])
                    nc.sync.dma_start(ctable[l][R1:R1 + H2 // 2, :],
                                      bounceB[l][:])
                else:
                    # two half-collectives: the first starts as soon as the
                    # first H1 node rows are normed, overlapping the tail of
                    # this layer's LN / transpose work
                    nc.gpsimd.collective_compute(
                        "AllGather", AluOp.bypass,
                        replica_groups=[list(range(NCORES))],
                        ins=[bounceA[l][:].opt()],
                        outs=[ctable[l][0:R1, :].opt()],
                    )
                    nc.gpsimd.collective_compute(
                        "AllGather", AluOp.bypass,
                        replica_groups=[list(range(NCORES))],
                        ins=[bounceB[l][:].opt()],
                        outs=[ctable[l][R1:N // 2, :].opt()],
                    )

            for l in range(L):
                build_table(l)

                # ---- gather per-edge pair-rows (one stream per parity) ----
                msgsA, msgsB = [], []
                for (T_p, msgs_l, pool, coff) in (
                    (TA, msgsA, msgpA, 0),
                    (TB, msgsB, msgpB, TA * 8),
                ):
                    for b in range(_ceil(T_p, BATCH_TILES)):
                        t0 = b * BATCH_TILES
                        bt = min(BATCH_TILES, T_p - t0)
                        mbuf = pool.tile([128, BATCH_TILES, 2 * O], f16,
                                         tag="msg", name="mbuf")
                        if DBG_NO_GATHER:
                            nc.vector.memset(mbuf[:, :bt, :], 0.5)
                        else:
                            nc.gpsimd.dma_gather(
                                mbuf[:, :bt, :],
                                ctable[l][:],
                                idx_sb[:, coff + t0 * 8:coff + (t0 + bt) * 8],
                                bt * 128, bt * 128, 2 * O,
                            )
                        msgs_l.append((t0, mbuf))

                # ---- segment sums (seg-major in PSUM):
                #      ps[s, f] = sum_e [rel_e == s] * msg[e, f]
                tiA = tiB = 0
                for w in range(nwin):
                    ntA, ntB = int(tilesA[w]), int(tilesB[w])
                    nt = ntA + ntB
                    if nt == 0:
                        continue   # stays zero from the initial memset
                    ps = psw.tile([WINSEG, O], f32, tag="psw", name="ps")
                    k = 0
                    for (ti, ntp, msgs_l, toff, lo) in (
                        (tiA, ntA, msgsA, 0, 0),
                        (tiB, ntB, msgsB, TA, O),
                    ):
                        for j in range(ntp):
                            t = ti + j
                            t0, mbuf = msgs_l[t // BATCH_TILES]
                            tc_col = toff + t
                            sel = selp.tile([128, 128], f16, tag="sel",
                                            name="sel")
                            nc.vector.tensor_scalar(
                                out=sel[:], in0=iota16[:],
                                scalar1=relf[:, tc_col:tc_col + 1],
                                scalar2=None, op0=AluOp.is_equal,
                            )
                            nc.tensor.matmul(
                                ps[:], sel[:], mbuf[:, t - t0, lo:lo + O],
                                start=(k == 0), stop=(k == nt - 1),
                            )
                            k += 1
                    tiA += ntA
                    tiB += ntB
                    # mean denominators are a per-partition (=per-segment)
                    # scalar in this orientation; scale while draining PSUM
                    stt = denp.tile([WINSEG, O], f32, tag="stt", name="stt")
                    nc.vector.tensor_scalar(
                        out=stt[:], in0=ps[:], scalar1=invf[:, w:w + 1],
                        scalar2=None, op0=AluOp.mult)
                    # transpose into feature-major S_T
                    psT = pstr.tile([128, 128], f32, tag="ptr", name="psT")
                    nc.tensor.matmul(psT[:O, :], stt[:], ident[:],
                                     start=True, stop=True)
                    nc.scalar.activation(
                        S_T[:, w * WINSEG:(w + 1) * WINSEG], psT[:O, :],
                        Act.Copy)

                # ---- per-relation transform + root + bias + relu ----
                S_nr = S_T[:].rearrange("p (n r) -> p n r", r=NUM_REL)
                for (o, n) in chunks(n_own, 512):
                    ps = pspost.tile([O, 512], f32, tag="pspost", name="ps")
                    for r in range(NUM_REL):
                        ci = C_RGCNW + (l * NUM_REL + r) * O
                        nc.tensor.matmul(
                            ps[:, :n], konst[:, ci:ci + O], S_nr[:, o:o + n, r],
                            start=(r == 0), stop=False,
                        )
                    ci = C_ROOTW + l * O
                    nc.tensor.matmul(
                        ps[:, :n], konst[:, ci:ci + O], hT[:, o:o + n],
                        start=False, stop=True,
                    )
                    outTc = strp.tile([O, 512], f32, tag="outTc", name="outTc")
                    nc.scalar.activation(outTc[:, :n], ps[:, :n], Act.Relu,
                                         bias=konst[:, C_BIAST + l:C_BIAST + l + 1])

                    # ---- transpose to rows + LayerNorm (128-node subchunks) --
                    for (o2, n2) in chunks(n, 128):
                        ps2 = pstr.tile([128, 128], f32, tag="ptr",
                                        name="ps2")
                        nc.tensor.matmul(ps2[:n2, :O], outTc[:, o2:o2 + n2],
                                         ident[:O, :O], start=True, stop=True)
                        rows = rowp.tile([128, O], f32, tag="rows", name="rows")
                        musum = lnp.tile([128, 1], f32, tag="musum",
                                         name="musum")
                        nc.scalar.activation(rows[:n2, :], ps2[:n2, :O],
                                             Act.Copy, accum_out=musum[:n2, :])
                        mu = lnp.tile([128, 1], f32, tag="mu", name="mu")
                        nc.vector.tensor_scalar(out=mu[:n2], in0=musum[:n2],
                                                scalar1=1.0 / O, scalar2=None,
                                                op0=AluOp.mult)
                        xc = lnp.tile([128, O], f32, tag="xc", name="xc")
                        nc.vector.tensor_scalar(out=xc[:n2, :], in0=rows[:n2, :],
                                                scalar1=mu[:n2], scalar2=None,
                                                op0=AluOp.subtract)
                        sq = lnp.tile([128, O], f32, tag="sq", name="sq")
                        varsum = lnp.tile([128, 1], f32, tag="varsum",
                                          name="varsum")
                        nc.scalar.activation(sq[:n2, :], xc[:n2, :], Act.Square,
                                             accum_out=varsum[:n2, :])
                        std = lnp.tile([128, 1], f32, tag="std", name="std")
                        nc.scalar.activation(std[:n2], varsum[:n2], Act.Sqrt,
                                             scale=1.0 / O, bias=epscol[:n2])
                        rstd = lnp.tile([128, 1], f32, tag="rstd", name="rstd")
                        nc.vector.reciprocal(rstd[:n2], std[:n2])
                        hrow = rowp.tile([128, O], f32, tag="hrow", name="hrow")
                        nc.vector.scalar_tensor_tensor(
                            out=hrow[:n2, :], in0=xc[:n2, :], scalar=rstd[:n2],
                            in1=gammaB[:n2, :], op0=AluOp.mult, op1=AluOp.mult,
                        )
                        go = o + o2
                        if l == L - 1:
                            hfin = rowp.tile([128, O], f32, tag="hfin",
                                             name="hfin")
                            nc.vector.tensor_tensor(out=hfin[:n2, :],
                                                    in0=hrow[:n2, :],
                                                    in1=betaB[:n2, :],
                                                    op=AluOp.add)
                            rmax = lnp.tile([128, 1], f32, tag="rmax",
                                            name="rmax")
                            nc.vector.tensor_reduce(
                                rmax[:n2], hfin[:n2, :],
                                axis=mybir.AxisListType.X, op=AluOp.max,
                                apply_absolute_value=True)
                            nc.vector.tensor_scalar(out=rmax[:n2],
                                                    in0=rmax[:n2],
                                                    scalar1=1e-3, scalar2=None,
                                                    op0=AluOp.max)
                            qsc = lnp.tile([128, 1], f32, tag="qsc",
                                           name="qsc")
                            nc.vector.reciprocal(qsc[:n2], rmax[:n2])
                            nc.vector.tensor_scalar(out=qsc[:n2], in0=qsc[:n2],
                                                    scalar1=127.0, scalar2=None,
                                                    op0=AluOp.mult)
                            t2 = rowp.tile([128, O], f32, tag="t2", name="t2")
                            nc.vector.scalar_tensor_tensor(
                                out=t2[:n2, :], in0=hfin[:n2, :],
                                scalar=qsc[:n2], in1=magicT[:n2, :],
                                op0=AluOp.mult, op1=AluOp.add)
                            out8 = rowp.tile([128, O + 2], dt.int8, tag="out8",
                                             name="out8")
                            nc.vector.tensor_scalar(
                                out=out8[:n2, 0:O], in0=t2[:n2, :],
                                scalar1=12582912.0, scalar2=None,
                                op0=AluOp.subtract)
                            sc16 = lnp.tile([128, 1], f16, tag="sc16",
                                            name="sc16")
                            nc.vector.tensor_scalar(out=sc16[:n2],
                                                    in0=rmax[:n2],
                                                    scalar1=1.0 / 127.0,
                                                    scalar2=None,
                                                    op0=AluOp.mult)
                            nc.vector.tensor_scalar(
                                out=out8[:n2, O:O + 2],
                                in0=sc16[:n2].bitcast(dt.int8),
                                scalar1=0, scalar2=None,
                                op0=AluOp.bitwise_or)
                            nc.sync.dma_start(out_d[go:go + n2, :],
                                              out8[:n2, :])
                        else:
                            nc.vector.tensor_tensor(out=hrow[:n2, :],
                                                    in0=hrow[:n2, :],
                                                    in1=betaB[:n2, :],
                                                    op=AluOp.add)
                            psb = pstr.tile([128, 128], f32, tag="ptr",
                                            name="psb")
                            nc.tensor.matmul(psb[:O, :n2], hrow[:n2, :],
                                             ident[:n2, :n2],
                                             start=True, stop=True)
                            nc.scalar.activation(hT[:, go:go + n2],
                                                 psb[:O, :n2], Act.Copy)

    nc.compile()
    return nc


def _make_in_maps(inputs, meta, cores, denom_inv=None):
    x = np.asarray(inputs["x"], dtype=np.float32)
    N = x.shape[0]
    n_own = N // NCORES

    konst = np.zeros((O, CC), dtype=np.float32)
    konst[:IN_DIM, 0:O] = np.asarray(inputs["feat2c_W"], np.float32)
    konst[:, C_RGCNW:C_RGCNW + L * NUM_REL * O] = (
        np.asarray(inputs["rgcn_W"], np.float32)
        .transpose(2, 0, 1, 3).reshape(O, L * NUM_REL * O))
    konst[:, C_ROOTW:C_ROOTW + L * O] = (
        np.asarray(inputs["rgcn_root"], np.float32)
        .transpose(1, 0, 2).reshape(O, L * O))
    konst[:, C_BIAST:C_BIAST + L] = np.asarray(
        inputs["rgcn_bias"], np.float32).T
    konst[:, C_F2CB] = np.asarray(inputs["feat2c_b"], np.float32)
    konst[0, C_GAMMA:C_GAMMA + O] = np.asarray(inputs["ln_gamma"], np.float32)
    konst[0, C_BETA:C_BETA + O] = np.asarray(inputs["ln_beta"], np.float32)

    konst16 = konst.astype(np.float16)
    kreg = (konst16.view(np.int8).reshape(4, 16, 2 * CC)
            .transpose(1, 0, 2).reshape(16, 8 * CC))
    XCH = _ceil(n_own, 128)
    in_maps = []
    for c in range(NCORES):
        xi = np.zeros((128, XCH * IN_DIM), dtype=np.float16)
        xc = x[c * n_own:(c + 1) * n_own, :]
        for k in range(XCH):
            nrows = min(128, n_own - k * 128)
            xi[:nrows, k * IN_DIM:(k + 1) * IN_DIM] = (
                xc[k * 128:k * 128 + nrows, :].astype(np.float16))
        xreg = (xi.view(np.int8).reshape(8, 16, XCH * IN_DIM * 2)
                .transpose(1, 0, 2).reshape(16, 8 * XCH * IN_DIM * 2))
        in_maps.append({"graph": cores[c], "wts": kreg,
                        "xin": np.ascontiguousarray(xreg)})
    return in_maps


class _Executor:
    """Compile the Bass program to a persistent jitted callable once, then
    re-dispatch it per call (same mechanism as bass_utils.run_bass_kernel_spmd
    -> bass2jax.run_bass_via_pjrt, but without rebuilding the jax.jit closure
    every call, which forces a full retrace + neuronx recompile each time).
    Output buffers are donated; the previous call's (already fetched) outputs
    are recycled as the next call's backing store — the kernel writes every
    output element, so contents don't matter."""

    def __init__(self, nc, n_cores=NCORES):
        import jax
        from jax.experimental.shard_map import shard_map
        from jax.sharding import Mesh, NamedSharding, PartitionSpec

        from concourse import bass2jax
        import concourse.mybir as mybir

        bass2jax.install_neuronx_cc_hook()
        assert nc.dbg_addr is None, "build with debug=False"
        self._jax = jax
        self.nc = nc
        self.n_cores = n_cores
        partition_name = (nc.partition_id_tensor.name
                          if nc.partition_id_tensor else None)
        in_names, out_names, out_shapes, out_avals = [], [], [], []
        for alloc in nc.m.functions[0].allocations:
            if not isinstance(alloc, mybir.MemoryLocationSet):
                continue
            name = alloc.memorylocations[0].name
            if alloc.kind == "ExternalInput":
                if name != partition_name:
                    in_names.append(name)
            elif alloc.kind == "ExternalOutput":
                shape = tuple(alloc.tensor_shape)
                dtype = mybir.dt.np(alloc.dtype)
                out_names.append(name)
                out_shapes.append((shape, dtype))
                out_avals.append(jax.core.ShapedArray(shape, dtype))
        self.in_names, self.out_names = in_names, out_names
        self._out_shapes = out_shapes
        n_params, n_outs = len(in_names), len(out_names)
        all_in = list(in_names) + list(out_names)
        if partition_name:
            all_in.append(partition_name)

        def _body(*args):
            operands = list(args)
            if partition_name:
                operands.append(bass2jax.partition_id_tensor())
            return tuple(bass2jax._bass_exec_p.bind(
                *operands, out_avals=tuple(out_avals),
                in_names=tuple(all_in), out_names=tuple(out_names),
                lowering_input_output_aliases=(),
                sim_require_finite=True, sim_require_nnan=True, nc=nc))

        devices = jax.devices()[:n_cores]
        mesh = Mesh(np.asarray(devices), ("core",))
        self._jit = jax.jit(
            shard_map(_body, mesh=mesh,
                      in_specs=(PartitionSpec("core"),) * (n_params + n_outs),
                      out_specs=(PartitionSpec("core"),) * n_outs,
                      check_rep=False),
            donate_argnums=tuple(range(n_params, n_params + n_outs)),
            keep_unused=True)
        self._sharding = NamedSharding(mesh, PartitionSpec("core"))
        self._prev = None
        # graph topology and weights are static across serving calls: keep
        # them device-resident, skip re-upload when the caller passes the
        # same arrays (node features "xin" always ship per call)
        self._static_names = {"graph", "wts"}
        self._static_cache = {}

    def run(self, in_maps):
        ins = []
        for name in self.in_names:
            arrs = [np.asarray(m[name]) for m in in_maps]
            if name in self._static_names:
                ce = self._static_cache.get(name)
                if ce is not None and len(ce[0]) == len(arrs) and all(
                        a is b for a, b in zip(ce[0], arrs)):
                    ins.append(ce[1])
                    continue
                darr = self._jax.device_put(
                    np.concatenate(arrs, axis=0), self._sharding)
                self._static_cache[name] = (arrs, darr)
                ins.append(darr)
            else:
                ins.append(np.concatenate(arrs, axis=0))
        if self._prev is None:
            outs_backing = [
                self._jax.device_put(
                    np.zeros((self.n_cores * s[0], *s[1:]), d),
                    self._sharding)
                for (s, d) in self._out_shapes]
        else:
            outs_backing = self._prev
        outs = self._jit(*ins, *outs_backing)
        host = [np.asarray(o) for o in outs]
        self._prev = list(outs)
        return [
            {name: host[i].reshape(self.n_cores, *self._out_shapes[i][0])[c]
             for i, name in enumerate(self.out_names)}
            for c in range(self.n_cores)]


_EXEC_CACHE = {}


def _get_exec(meta):
    key = (meta["N"], meta["n_own"], meta["TA"], meta["TB"], meta["nwin"],
           meta["segp_pad"])
    ex = _EXEC_CACHE.get(key)
    if ex is None:
        ex = _Executor(_build_program(meta))
        _EXEC_CACHE[key] = ex
    return ex


class _Result:
    def __init__(self, results, exec_time_ns=None):
        self.results = results
        self.exec_time_ns = exec_time_ns


def _run(inputs, meta, cores, denom_inv=None, profile=False):
    import time

    ex = _get_exec(meta)
    in_maps = _make_in_maps(inputs, meta, cores)
    res = _Result(ex.run(in_maps))
    if profile:
        # no NTFF hook in this container: report min warm wall-clock of a
        # full dispatch (host input concat + tunnel transfer + device exec
        # + output fetch; upper bound on device time). The axon relay has
        # tens-of-ms jitter, so take the min over a larger sample.
        best = None
        for _ in range(20):
            t0 = time.time()
            res.results = ex.run(in_maps)
            dt = time.time() - t0
            best = dt if best is None else min(best, dt)
        res.exec_time_ns = int(best * 1e9)
    raw = np.concatenate([res.results[c]["out"] for c in range(NCORES)],
                         axis=0)
    q = raw[:, :O].astype(np.float32)
    sc = np.ascontiguousarray(raw[:, O:O + 2]).view(np.float16)
    out = q * sc.astype(np.float32)
    return out, res


def kernel(x, edge_index, edge_type, feat2c_W, feat2c_b, rgcn_W, rgcn_root,
           rgcn_bias, ln_gamma, ln_beta):
    inputs = dict(x=x, edge_index=edge_index, edge_type=edge_type,
                  feat2c_W=feat2c_W, feat2c_b=feat2c_b, rgcn_W=rgcn_W,
                  rgcn_root=rgcn_root, rgcn_bias=rgcn_bias,
                  ln_gamma=ln_gamma, ln_beta=ln_beta)
    meta, cores, _ = _preprocess(
        np.asarray(x), np.asarray(edge_index), np.asarray(edge_type))
    out, _ = _run(inputs, meta, cores, profile=False)
    return out


if __name__ == "__main__":
    pass



# revision 114
# speedup vs baseline: 1.3789x; 1.3789x over previous
"""Trainium2 Bass kernel for DynamicGNN (3-layer RGCN-style message passing).

Strategy: shard destination nodes (and their incoming edges) across the 8
NeuronCores. Each core owns N/8 nodes = 3*N/8 (node,relation) segments.
Messages are gathered per-edge from a replicated f16 PAIR-ROW node table
([N/2, 128] f16: two nodes share one 256B dma_gather row; idx = pair_row,
and tiles are split per window into even-src / odd-src classes so the
valid half is a compile-time AP slice). Segment reduction runs on the
TensorEngine: per 128-edge tile a f16 selection mask (iota==rel) is
loaded stationary and the f16 messages stream, so PSUM accumulates
[segment, feature]; mean denominators are a per-partition scalar multiply
on the PSUM drain, and one transpose matmul per 128-segment window lands
the result feature-major in S_T. The per-layer table rebuild AllGathers
compact f16 rows directly into the pair-row table in two halves (no
local expand pass). Per-relation transform/root/bias/relu/LayerNorm run
feature-major in chunks of 512 nodes.

Host->device traffic is kept minimal: per core a static "graph" blob
([16, X] int8: compact int16 gather indices, int8 per-slot window-local
segment ids, f16 per-segment inverse counts), a "wts" blob (f16 weights)
and a per-call "xin" blob (f16 x row-image); all replicated/bitcast/
converted on device. The compiled executable is cached per graph shape,
and graph/wts device buffers are cached across calls that pass identical
arrays (serving mode); output buffers are donated back each call.
Output is int8 with a per-row f16 scale packed in the same tensor
(dequantized on host)."""
import os
import sys

import numpy as np

DBG_NO_GATHER = os.environ.get("DBG_NO_GATHER") == "1"
DBG_NO_COLLECTIVE = os.environ.get("DBG_NO_COLLECTIVE") == "1"

sys.path.insert(0, "/opt/trn_rl_repo")

NCORES = 8
WINSEG = 128          # segments per psum window
BATCH_TILES = 8       # 128-edge tiles per dma_gather call (desc-ring limit ~1024)
LN_EPS = 1e-5
NUM_REL = 3

O = 64
L = 3
IN_DIM = 5

# const array column layout ([64, CC] f32)
C_F2CW = 0            # rows 0:5, cols 0:64
C_RGCNW = 64          # rows 0:64, 576 cols ((l*3+r)*64)
C_ROOTW = 640         # rows 0:64, 192 cols
C_BIAST = 832         # rows 0:64, 3 cols
C_F2CB = 835          # rows 0:64, 1 col
C_GAMMA = 836         # row 0, 64 cols
C_BETA = 900          # row 0, 64 cols
CC = 964


def _ceil(a, b):
    return (a + b - 1) // b


def _preprocess(x, edge_index, edge_type):
    """Host-side: shard edges by dst owner, sort by segment, build windows,
    per-phase slot streams (gather idx + per-slot rel/count metadata)."""
    N = x.shape[0]
    E = edge_index.shape[1]
    n_own = N // NCORES
    seg_per_core = n_own * NUM_REL
    nwin = _ceil(seg_per_core, WINSEG)

    src = edge_index[0].astype(np.int64)
    dst = edge_index[1].astype(np.int64)
    et = edge_type.astype(np.int64)

    assert n_own % 2 == 0, "pair-row gather table needs even n_own"
    # the per-layer AllGather is split in two so the first half overlaps the
    # tail of the previous stage; table layout = [all cores' first H1 nodes |
    # all cores' last H2 nodes], both halves even-sized for pair rows
    H1 = max(128, (n_own // 2 // 128) * 128)   # 128-aligned, even
    H2 = n_own - H1
    R1 = NCORES * (H1 // 2)        # pair rows in region 1

    def pair_row(s):
        c, p = s // n_own, s % n_own
        return np.where(p < H1, c * (H1 // 2) + (p >> 1),
                        R1 + c * (H2 // 2) + ((p - H1) >> 1))

    owner = dst // n_own
    cntE = np.zeros((NCORES, nwin), dtype=np.int64)
    cntO = np.zeros((NCORES, nwin), dtype=np.int64)
    per_core = []
    for c in range(NCORES):
        m = owner == c
        s_c = src[m]
        seg_c = (dst[m] - c * n_own) * NUM_REL + et[m]
        # sort by (segment, src): src-sorted runs compress better on the wire
        order = np.lexsort((s_c, seg_c))
        s_c = s_c[order]
        seg_c = seg_c[order]
        w_c = seg_c // WINSEG
        # parity classes: even-src tiles read the low half of the pair row,
        # odd-src tiles the high half (compile-time AP slice, no merge ops)
        isE = (s_c & 1) == 0
        cntE[c] = np.bincount(w_c[isE], minlength=nwin)
        cntO[c] = np.bincount(w_c[~isE], minlength=nwin)
        # per-(local segment) counts for mean denominators
        segcnt = np.bincount(seg_c, minlength=seg_per_core)
        per_core.append((s_c, seg_c, w_c, isE, segcnt))

    # compile-time tile structure: tiles per (parity, window) = max over cores
    tilesA = _ceil(np.maximum(cntE.max(axis=0), 0), 128)   # [nwin]
    tilesB = _ceil(np.maximum(cntO.max(axis=0), 0), 128)
    TA, TB = int(tilesA.sum()), int(tilesB.sum())
    T = TA + TB
    slotsA = TA * 128
    tbaseA = np.concatenate([[0], np.cumsum(tilesA)[:-1]])
    tbaseB = np.concatenate([[0], np.cumsum(tilesB)[:-1]])

    cores = []
    for c in range(NCORES):
        s_c, seg_c, w_c, isE, segcnt = per_core[c]
        idx_s = np.zeros(T * 128, dtype=np.int16)
        rel_s = np.full(T * 128, -1, dtype=np.int8)
        for (mask, tbase, soff) in ((isE, tbaseA, 0), (~isE, tbaseB, slotsA)):
            s_p = s_c[mask]
            seg_p = seg_c[mask]
            w_p = w_c[mask]
            gc = np.bincount(w_p, minlength=nwin)
            starts = np.concatenate([[0], np.cumsum(gc)[:-1]])
            pos = np.arange(len(w_p)) - starts[w_p]
            slot = soff + tbase[w_p] * 128 + pos
            idx_s[slot] = pair_row(s_p).astype(np.int16)
            rel_s[slot] = (seg_p - w_p * WINSEG).astype(np.int8)
        # per-segment inverse counts (mean denominators), f16 on the wire,
        # laid out [seg-in-window, window] for the device
        inv = np.ones(nwin * WINSEG, dtype=np.float16)
        inv[:seg_per_core] = (
            1.0 / np.maximum(segcnt, 1).astype(np.float64)).astype(np.float16)
        invT = np.ascontiguousarray(inv.reshape(nwin, WINSEG).T)  # [128,nwin]
        inv16 = (invT.view(np.int8).reshape(8, 16, 2 * nwin)
                 .transpose(1, 0, 2).reshape(16, 16 * nwin))

        # idx: [S] -> [16, S/16]; rel: [S] -> [128, S/128] -> regroup
        relp = rel_s.reshape(-1, 128).T
        rel16 = relp.reshape(8, 16, T).transpose(1, 0, 2).reshape(16, 8 * T)
        idx16 = np.ascontiguousarray(idx_s.reshape(-1, 16).T)
        blob = np.concatenate([idx16.view(np.int8), rel16, inv16], axis=1)
        cores.append(np.ascontiguousarray(blob))

    segp_pad = _ceil(nwin * WINSEG, 3 * 512) * (3 * 512)
    meta = dict(
        N=N, E=E, n_own=n_own, seg_per_core=seg_per_core, nwin=nwin,
        tilesA=tilesA, tilesB=tilesB, TA=TA, TB=TB, segp_pad=segp_pad,
        H1=H1, H2=H2,
    )
    return meta, cores, None


def _build_program(meta):
    import concourse.bacc as bacc
    import concourse.mybir as mybir
    import concourse.tile as tile
    from concourse.masks import make_identity

    dt = mybir.dt
    f32 = dt.float32
    f16 = dt.float16
    N = meta["N"]
    n_own = meta["n_own"]
    nwin = meta["nwin"]
    tilesA, tilesB = meta["tilesA"], meta["tilesB"]
    TA, TB = meta["TA"], meta["TB"]
    T = TA + TB
    segp_pad = meta["segp_pad"]
    H1, H2 = meta["H1"], meta["H2"]
    R1 = NCORES * (H1 // 2)

    nc = bacc.Bacc("TRN2", target_bir_lowering=False, debug=False,
                   enable_asserts=False, num_devices=NCORES)

    IOFF = 24 * T                          # inverse-count region byte offset
    IVR = 2 * nwin                         # inv bytes per 16-row group
    IVB = 8 * IVR                          # inv bytes per row
    XCH = _ceil(n_own, 128)                # 128-node chunks of x
    graph_d = nc.dram_tensor("graph", [16, IOFF + IVB], dt.int8,
                             kind="ExternalInput")
    wts_d = nc.dram_tensor("wts", [16, 8 * CC], dt.int8,
                           kind="ExternalInput")
    xin_d = nc.dram_tensor("xin", [16, 8 * XCH * IN_DIM * 2], dt.int8,
                           kind="ExternalInput")
    out_d = nc.dram_tensor("out", [n_own, O + 2], dt.int8,
                           kind="ExternalOutput")

    AluOp = mybir.AluOpType
    Act = mybir.ActivationFunctionType

    with tile.TileContext(nc) as tc:
        with (
            tc.tile_pool(name="persist", bufs=1) as pp,
            tc.tile_pool(name="msgpA", bufs=5) as msgpA,
            tc.tile_pool(name="msgpB", bufs=5) as msgpB,
            tc.tile_pool(name="selp", bufs=8) as selp,
            tc.tile_pool(name="denp", bufs=2) as denp,
            tc.tile_pool(name="rowp", bufs=6) as rowp,
            tc.tile_pool(name="lnp", bufs=8) as lnp,
            tc.tile_pool(name="strp", bufs=3) as strp,
            tc.tile_pool(name="psw", bufs=2, space="PSUM") as psw,
            tc.tile_pool(name="pspost", bufs=2, space="PSUM") as pspost,
            tc.tile_pool(name="pstr", bufs=2, space="PSUM") as pstr,
            tc.tile_pool(name="pstb", bufs=1, space="PSUM") as pstb,
            tc.tile_pool(name="psd", bufs=1, space="PSUM") as psd,
            tc.tile_pool(name="dram", bufs=1, space="DRAM") as dr,
        ):
            def persist(name, shape, d=f32):
                return pp.tile(shape, d, tag=name, name=name)

            idx_sb = persist("idx_sb", [128, T * 8], dt.int16)
            rel8 = persist("rel8", [128, T], dt.int8)
            konst16 = persist("konst16", [O, CC], f16)
            ximg = persist("ximg", [128, XCH * IN_DIM], f16)
            ximg32 = persist("ximg32", [128, XCH * IN_DIM])
            magicT = persist("magicT", [128, O])
            relf = persist("relf", [128, T])
            inv16 = persist("inv16", [128, nwin], f16)
            invf = persist("invf", [128, nwin])
            iota = persist("iota", [128, 128])
            iota16 = persist("iota16", [128, 128], f16)
            ident = persist("ident", [128, 128])
            konst = persist("konst", [O, CC])
            gammaB = persist("gammaB", [128, O])
            betaB = persist("betaB", [128, O])
            onesrow = persist("onesrow", [1, 128])
            S_T = persist("S_T", [O, segp_pad])
            hT = persist("hT", [O, n_own])
            epscol = persist("epscol", [128, 1])

            idx_src = graph_d[:, 0:16 * T].bitcast(dt.int16)
            for g in range(8):
                nc.sync.dma_start(idx_sb[g * 16:(g + 1) * 16, :], idx_src)
                nc.sync.dma_start(
                    rel8[g * 16:(g + 1) * 16, :],
                    graph_d[:, 16 * T + g * T:16 * T + (g + 1) * T])
            for g in range(8):
                nc.sync.dma_start(
                    inv16[g * 16:(g + 1) * 16, :],
                    graph_d[:, IOFF + g * IVR:IOFF + (g + 1) * IVR]
                    .bitcast(f16))
            nc.scalar.activation(invf[:], inv16[:], Act.Copy)
            for g in range(4):
                nc.sync.dma_start(
                    konst16[g * 16:(g + 1) * 16, :],
                    wts_d[:, g * 2 * CC:(g + 1) * 2 * CC].bitcast(f16))
            nc.scalar.activation(konst[:], konst16[:], Act.Copy)
            XR = XCH * IN_DIM * 2
            for g in range(8):
                nc.sync.dma_start(
                    ximg[g * 16:(g + 1) * 16, :],
                    xin_d[:, g * XR:(g + 1) * XR].bitcast(f16))
            nc.scalar.activation(ximg32[:], ximg[:], Act.Copy)
            nc.vector.memset(magicT[:], 12582912.0)

            make_identity(nc, ident[:])
            nc.gpsimd.iota(iota[:], [[1, 128]], channel_multiplier=0,
                           allow_small_or_imprecise_dtypes=True)
            nc.scalar.activation(iota16[:], iota[:], Act.Copy)
            nc.vector.memset(epscol[:], LN_EPS)
            nc.vector.memset(onesrow[:], 1.0)
            nc.vector.memset(S_T[:], 0.0)

            # decode slot metadata: rel (f32)
            nc.vector.tensor_scalar(out=relf[:], in0=rel8[:],
                                    scalar1=0.0, scalar2=None, op0=AluOp.add)

            # broadcast gamma/beta rows across 128 partitions via ones-matmul
            for (col, dst) in ((C_GAMMA, gammaB), (C_BETA, betaB)):
                psg = pstr.tile([128, 128], f32, tag="ptr", name="psg")
                nc.tensor.matmul(psg[:, :O], onesrow[:],
                                 konst[0:1, col:col + O], start=True, stop=True)
                nc.scalar.activation(dst[:], psg[:, :O], Act.Copy)

            # DRAM internals: per-layer bounce -> pair-row AllGather target.
            # dma_gather rows must be 256B-aligned, so the gather table packs
            # TWO nodes per row ([N/2, 128] f16); idx = src>>1 and the valid
            # half is selected per edge on the vector engine (parity merge).
            bounceA = [dr.tile([H1, O], f16, tag=f"bounceA{l}",
                               name=f"bounceA{l}") for l in range(L)]
            bounceB = [dr.tile([H2, O], f16, tag=f"bounceB{l}",
                               name=f"bounceB{l}") for l in range(L)]
            ctable = [dr.tile([N // 2, 2 * O], f16, tag=f"ctable{l}",
                              name=f"ctable{l}") for l in range(L)]

            def chunks(total, step):
                return [(i, min(step, total - i)) for i in range(0, total, step)]

            # ---- layer 0 node features: h0T = f2cW.T @ xT (+bias) ----
            ximg3 = ximg32[:].rearrange("p (k d) -> p k d", d=IN_DIM)
            for ki, (o, n2) in enumerate(chunks(n_own, 128)):
                psX = pstr.tile([128, 128], f32, tag="ptr", name="psX")
                nc.tensor.matmul(psX[:IN_DIM, :n2], ximg3[:n2, ki, :],
                                 ident[:n2, :n2], start=True, stop=True)
                xTc = strp.tile([IN_DIM, 128], f32, tag="xTc", name="xTc")
                nc.scalar.activation(xTc[:, :n2], psX[:IN_DIM, :n2], Act.Copy)
                ps = pspost.tile([O, 512], f32, tag="pspost", name="ps")
                nc.tensor.matmul(ps[:, :n2], konst[0:IN_DIM, 0:O], xTc[:, :n2],
                                 start=True, stop=True)
                nc.scalar.activation(hT[:, o:o + n2], ps[:, :n2], Act.Identity,
                                     bias=konst[:, C_F2CB:C_F2CB + 1])

            def build_table(l):
                """transpose hT columns into f16 row chunks, DMA to the two
                bounce halves, AllGather each into the pair-row table."""
                for (o, n) in chunks(n_own, 128):
                    ps = pstb.tile([128, 128], f32, tag="ptb", name="ps")
                    nc.tensor.matmul(ps[:n, :O], hT[:, o:o + n], ident[:O, :O],
                                     start=True, stop=True)
                    rows = rowp.tile([128, O], f16, tag="rows16",
                                     name="rows")
                    nc.scalar.activation(rows[:n, :], ps[:n, :O], Act.Copy)
                    if o < H1:
                        nc.sync.dma_start(bounceA[l][o:o + n, :], rows[:n, :])
                    else:
                        nc.sync.dma_start(bounceB[l][o - H1:o - H1 + n, :],
                                          rows[:n, :])
                if DBG_NO_COLLECTIVE:
                    nc.sync.dma_start(ctable[l][0:H1 // 2, :], bounceA[l][:])
                    nc.sync.dma_start(ctable[l][R1:R1 + H2 // 2, :],
                                      bounceB[l][:])
                else:
                    # two half-collectives: the first starts as soon as the
                    # first H1 node rows are normed, overlapping the tail of
                    # this layer's LN / transpose work
                    nc.gpsimd.collective_compute(
                        "AllGather", AluOp.bypass,
                        replica_groups=[list(range(NCORES))],
                        ins=[bounceA[l][:].opt()],
                        outs=[ctable[l][0:R1, :].opt()],
                    )
                    nc.gpsimd.collective_compute(
                        "AllGather", AluOp.bypass,
                        replica_groups=[list(range(NCORES))],
                        ins=[bounceB[l][:].opt()],
                        outs=[ctable[l][R1:N // 2, :].opt()],
                    )

            for l in range(L):
                build_table(l)

                # ---- gather per-edge pair-rows (one stream per parity) ----
                msgsA, msgsB = [], []
                for (T_p, msgs_l, pool, coff) in (
                    (TA, msgsA, msgpA, 0),
                    (TB, msgsB, msgpB, TA * 8),
                ):
                    for b in range(_ceil(T_p, BATCH_TILES)):
                        t0 = b * BATCH_TILES
                        bt = min(BATCH_TILES, T_p - t0)
                        mbuf = pool.tile([128, BATCH_TILES, 2 * O], f16,
                                         tag="msg", name="mbuf")
                        if DBG_NO_GATHER:
                            nc.vector.memset(mbuf[:, :bt, :], 0.5)
                        else:
                            nc.gpsimd.dma_gather(
                                mbuf[:, :bt, :],
                                ctable[l][:],
                                idx_sb[:, coff + t0 * 8:coff + (t0 + bt) * 8],
                                bt * 128, bt * 128, 2 * O,
                            )
                        msgs_l.append((t0, mbuf))

                # ---- segment sums (seg-major in PSUM):
                #      ps[s, f] = sum_e [rel_e == s] * msg[e, f]
                tiA = tiB = 0
                for w in range(nwin):
                    ntA, ntB = int(tilesA[w]), int(tilesB[w])
                    nt = ntA + ntB
                    if nt == 0:
                        continue   # stays zero from the initial memset
                    ps = psw.tile([WINSEG, O], f32, tag="psw", name="ps")
                    k = 0
                    for (ti, ntp, msgs_l, toff, lo) in (
                        (tiA, ntA, msgsA, 0, 0),
                        (tiB, ntB, msgsB, TA, O),
                    ):
                        for j in range(ntp):
                            t = ti + j
                            t0, mbuf = msgs_l[t // BATCH_TILES]
                            tc_col = toff + t
                            sel = selp.tile([128, 128], f16, tag="sel",
                                            name="sel")
                            nc.vector.tensor_scalar(
                                out=sel[:], in0=iota16[:],
                                scalar1=relf[:, tc_col:tc_col + 1],
                                scalar2=None, op0=AluOp.is_equal,
                            )
                            nc.tensor.matmul(
                                ps[:], sel[:], mbuf[:, t - t0, lo:lo + O],
                                start=(k == 0), stop=(k == nt - 1),
                            )
                            k += 1
                    tiA += ntA
                    tiB += ntB
                    # mean denominators are a per-partition (=per-segment)
                    # scalar in this orientation; scale while draining PSUM
                    stt = denp.tile([WINSEG, O], f32, tag="stt", name="stt")
                    nc.vector.tensor_scalar(
                        out=stt[:], in0=ps[:], scalar1=invf[:, w:w + 1],
                        scalar2=None, op0=AluOp.mult)
                    # transpose into feature-major S_T
                    psT = psd.tile([128, 128], f32, tag="psd", name="psT")
                    nc.tensor.matmul(psT[:O, :], stt[:], ident[:],
                                     start=True, stop=True)
                    nc.scalar.activation(
                        S_T[:, w * WINSEG:(w + 1) * WINSEG], psT[:O, :],
                        Act.Copy)

                # ---- per-relation transform + root + bias + relu ----
                S_nr = S_T[:].rearrange("p (n r) -> p n r", r=NUM_REL)
                for (o, n) in chunks(n_own, 512):
                    ps = pspost.tile([O, 512], f32, tag="pspost", name="ps")
                    for r in range(NUM_REL):
                        ci = C_RGCNW + (l * NUM_REL + r) * O
                        nc.tensor.matmul(
                            ps[:, :n], konst[:, ci:ci + O], S_nr[:, o:o + n, r],
                            start=(r == 0), stop=False,
                        )
                    ci = C_ROOTW + l * O
                    nc.tensor.matmul(
                        ps[:, :n], konst[:, ci:ci + O], hT[:, o:o + n],
                        start=False, stop=True,
                    )
                    outTc = strp.tile([O, 512], f32, tag="outTc", name="outTc")
                    nc.scalar.activation(outTc[:, :n], ps[:, :n], Act.Relu,
                                         bias=konst[:, C_BIAST + l:C_BIAST + l + 1])

                    # ---- transpose to rows + LayerNorm (128-node subchunks) --
                    for (o2, n2) in chunks(n, 128):
                        ps2 = pstr.tile([128, 128], f32, tag="ptr",
                                        name="ps2")
                        nc.tensor.matmul(ps2[:n2, :O], outTc[:, o2:o2 + n2],
                                         ident[:O, :O], start=True, stop=True)
                        rows = rowp.tile([128, O], f32, tag="rows", name="rows")
                        musum = lnp.tile([128, 1], f32, tag="musum",
                                         name="musum")
                        nc.scalar.activation(rows[:n2, :], ps2[:n2, :O],
                                             Act.Copy, accum_out=musum[:n2, :])
                        mu = lnp.tile([128, 1], f32, tag="mu", name="mu")
                        nc.vector.tensor_scalar(out=mu[:n2], in0=musum[:n2],
                                                scalar1=1.0 / O, scalar2=None,
                                                op0=AluOp.mult)
                        xc = lnp.tile([128, O], f32, tag="xc", name="xc")
                        nc.vector.tensor_scalar(out=xc[:n2, :], in0=rows[:n2, :],
                                                scalar1=mu[:n2], scalar2=None,
                                                op0=AluOp.subtract)
                        sq = lnp.tile([128, O], f32, tag="sq", name="sq")
                        varsum = lnp.tile([128, 1], f32, tag="varsum",
                                          name="varsum")
                        nc.scalar.activation(sq[:n2, :], xc[:n2, :], Act.Square,
                                             accum_out=varsum[:n2, :])
                        std = lnp.tile([128, 1], f32, tag="std", name="std")
                        nc.scalar.activation(std[:n2], varsum[:n2], Act.Sqrt,
                                             scale=1.0 / O, bias=epscol[:n2])
                        rstd = lnp.tile([128, 1], f32, tag="rstd", name="rstd")
                        nc.vector.reciprocal(rstd[:n2], std[:n2])
                        hrow = rowp.tile([128, O], f32, tag="hrow", name="hrow")
                        nc.vector.scalar_tensor_tensor(
                            out=hrow[:n2, :], in0=xc[:n2, :], scalar=rstd[:n2],
                            in1=gammaB[:n2, :], op0=AluOp.mult, op1=AluOp.mult,
                        )
                        go = o + o2
                        if l == L - 1:
                            hfin = rowp.tile([128, O], f32, tag="hfin",
                                             name="hfin")
                            nc.vector.tensor_tensor(out=hfin[:n2, :],
                                                    in0=hrow[:n2, :],
                                                    in1=betaB[:n2, :],
                                                    op=AluOp.add)
                            rmax = lnp.tile([128, 1], f32, tag="rmax",
                                            name="rmax")
                            nc.vector.tensor_reduce(
                                rmax[:n2], hfin[:n2, :],
                                axis=mybir.AxisListType.X, op=AluOp.max,
                                apply_absolute_value=True)
                            nc.vector.tensor_scalar(out=rmax[:n2],
                                                    in0=rmax[:n2],
                                                    scalar1=1e-3, scalar2=None,
                                                    op0=AluOp.max)
                            qsc = lnp.tile([128, 1], f32, tag="qsc",
                                           name="qsc")
                            nc.vector.reciprocal(qsc[:n2], rmax[:n2])
                            nc.vector.tensor_scalar(out=qsc[:n2], in0=qsc[:n2],
                                                    scalar1=127.0, scalar2=None,
                                                    op0=AluOp.mult)
                            t2 = rowp.tile([128, O], f32, tag="t2", name="t2")
                            nc.vector.scalar_tensor_tensor(
                                out=t2[:n2, :], in0=hfin[:n2, :],
                                scalar=qsc[:n2], in1=magicT[:n2, :],
                                op0=AluOp.mult, op1=AluOp.add)
                            out8 = rowp.tile([128, O + 2], dt.int8, tag="out8",
                                             name="out8")
                            nc.vector.tensor_scalar(
                                out=out8[:n2, 0:O], in0=t2[:n2, :],
                                scalar1=12582912.0, scalar2=None,
                                op0=AluOp.subtract)
                            sc16 = lnp.tile([128, 1], f16, tag="sc16",
                                            name="sc16")
                            nc.vector.tensor_scalar(out=sc16[:n2],
                                                    in0=rmax[:n2],
                                                    scalar1=1.0 / 127.0,
                                                    scalar2=None,
                                                    op0=AluOp.mult)
                            nc.vector.tensor_scalar(
                                out=out8[:n2, O:O + 2],
                                in0=sc16[:n2].bitcast(dt.int8),
                                scalar1=0, scalar2=None,
                                op0=AluOp.bitwise_or)
                            nc.sync.dma_start(out_d[go:go + n2, :],
                                              out8[:n2, :])
                        else:
                            nc.vector.tensor_tensor(out=hrow[:n2, :],
                                                    in0=hrow[:n2, :],
                                                    in1=betaB[:n2, :],
                                                    op=AluOp.add)
                            psb = pstr.tile([128, 128], f32, tag="ptr",
                                            name="psb")
                            nc.tensor.matmul(psb[:O, :n2], hrow[:n2, :],
                                             ident[:n2, :n2],
                                             start=True, stop=True)
                            nc.scalar.activation(hT[:, go:go + n2],
                                                 psb[:O, :n2], Act.Copy)

    nc.compile()
    return nc


def _make_in_maps(inputs, meta, cores, denom_inv=None):
    x = np.asarray(inputs["x"], dtype=np.float32)
    N = x.shape[0]
    n_own = N // NCORES

    konst = np.zeros((O, CC), dtype=np.float32)
    konst[:IN_DIM, 0:O] = np.asarray(inputs["feat2c_W"], np.float32)
    konst[:, C_RGCNW:C_RGCNW + L * NUM_REL * O] = (
        np.asarray(inputs["rgcn_W"], np.float32)
        .transpose(2, 0, 1, 3).reshape(O, L * NUM_REL * O))
    konst[:, C_ROOTW:C_ROOTW + L * O] = (
        np.asarray(inputs["rgcn_root"], np.float32)
        .transpose(1, 0, 2).reshape(O, L * O))
    konst[:, C_BIAST:C_BIAST + L] = np.asarray(
        inputs["rgcn_bias"], np.float32).T
    konst[:, C_F2CB] = np.asarray(inputs["feat2c_b"], np.float32)
    konst[0, C_GAMMA:C_GAMMA + O] = np.asarray(inputs["ln_gamma"], np.float32)
    konst[0, C_BETA:C_BETA + O] = np.asarray(inputs["ln_beta"], np.float32)

    konst16 = konst.astype(np.float16)
    kreg = (konst16.view(np.int8).reshape(4, 16, 2 * CC)
            .transpose(1, 0, 2).reshape(16, 8 * CC))
    XCH = _ceil(n_own, 128)
    in_maps = []
    for c in range(NCORES):
        xi = np.zeros((128, XCH * IN_DIM), dtype=np.float16)
        xc = x[c * n_own:(c + 1) * n_own, :]
        for k in range(XCH):
            nrows = min(128, n_own - k * 128)
            xi[:nrows, k * IN_DIM:(k + 1) * IN_DIM] = (
                xc[k * 128:k * 128 + nrows, :].astype(np.float16))
        xreg = (xi.view(np.int8).reshape(8, 16, XCH * IN_DIM * 2)
                .transpose(1, 0, 2).reshape(16, 8 * XCH * IN_DIM * 2))
        in_maps.append({"graph": cores[c], "wts": kreg,
                        "xin": np.ascontiguousarray(xreg)})
    return in_maps


class _Executor:
    """Compile the Bass program to a persistent jitted callable once, then
    re-dispatch it per call (same mechanism as bass_utils.run_bass_kernel_spmd
    -> bass2jax.run_bass_via_pjrt, but without rebuilding the jax.jit closure
    every call, which forces a full retrace + neuronx recompile each time).
    Output buffers are donated; the previous call's (already fetched) outputs
    are recycled as the next call's backing store — the kernel writes every
    output element, so contents don't matter."""

    def __init__(self, nc, n_cores=NCORES):
        import jax
        from jax.experimental.shard_map import shard_map
        from jax.sharding import Mesh, NamedSharding, PartitionSpec

        from concourse import bass2jax
        import concourse.mybir as mybir

        bass2jax.install_neuronx_cc_hook()
        assert nc.dbg_addr is None, "build with debug=False"
        self._jax = jax
        self.nc = nc
        self.n_cores = n_cores
        partition_name = (nc.partition_id_tensor.name
                          if nc.partition_id_tensor else None)
        in_names, out_names, out_shapes, out_avals = [], [], [], []
        for alloc in nc.m.functions[0].allocations:
            if not isinstance(alloc, mybir.MemoryLocationSet):
                continue
            name = alloc.memorylocations[0].name
            if alloc.kind == "ExternalInput":
                if name != partition_name:
                    in_names.append(name)
            elif alloc.kind == "ExternalOutput":
                shape = tuple(alloc.tensor_shape)
                dtype = mybir.dt.np(alloc.dtype)
                out_names.append(name)
                out_shapes.append((shape, dtype))
                out_avals.append(jax.core.ShapedArray(shape, dtype))
        self.in_names, self.out_names = in_names, out_names
        self._out_shapes = out_shapes
        n_params, n_outs = len(in_names), len(out_names)
        all_in = list(in_names) + list(out_names)
        if partition_name:
            all_in.append(partition_name)

        def _body(*args):
            operands = list(args)
            if partition_name:
                operands.append(bass2jax.partition_id_tensor())
            return tuple(bass2jax._bass_exec_p.bind(
                *operands, out_avals=tuple(out_avals),
                in_names=tuple(all_in), out_names=tuple(out_names),
                lowering_input_output_aliases=(),
                sim_require_finite=True, sim_require_nnan=True, nc=nc))

        devices = jax.devices()[:n_cores]
        mesh = Mesh(np.asarray(devices), ("core",))
        self._jit = jax.jit(
            shard_map(_body, mesh=mesh,
                      in_specs=(PartitionSpec("core"),) * (n_params + n_outs),
                      out_specs=(PartitionSpec("core"),) * n_outs,
                      check_rep=False),
            donate_argnums=tuple(range(n_params, n_params + n_outs)),
            keep_unused=True)
        self._sharding = NamedSharding(mesh, PartitionSpec("core"))
        self._prev = None
        # graph topology and weights are static across serving calls: keep
        # them device-resident, skip re-upload when the caller passes the
        # same arrays (node features "xin" always ship per call)
        self._static_names = {"graph", "wts"}
        self._static_cache = {}

    def run(self, in_maps):
        ins = []
        for name in self.in_names:
            arrs = [np.asarray(m[name]) for m in in_maps]
            if name in self._static_names:
                ce = self._static_cache.get(name)
                if ce is not None and len(ce[0]) == len(arrs) and all(
                        a is b for a, b in zip(ce[0], arrs)):
                    ins.append(ce[1])
                    continue
                darr = self._jax.device_put(
                    np.concatenate(arrs, axis=0), self._sharding)
                self._static_cache[name] = (arrs, darr)
                ins.append(darr)
            else:
                ins.append(np.concatenate(arrs, axis=0))
        if self._prev is None:
            outs_backing = [
                self._jax.device_put(
                    np.zeros((self.n_cores * s[0], *s[1:]), d),
                    self._sharding)
                for (s, d) in self._out_shapes]
        else:
            outs_backing = self._prev
        outs = self._jit(*ins, *outs_backing)
        host = [np.asarray(o) for o in outs]
        self._prev = list(outs)
        return [
            {name: host[i].reshape(self.n_cores, *self._out_shapes[i][0])[c]
             for i, name in enumerate(self.out_names)}
            for c in range(self.n_cores)]


_EXEC_CACHE = {}


def _get_exec(meta):
    key = (meta["N"], meta["n_own"], meta["TA"], meta["TB"], meta["nwin"],
           meta["segp_pad"])
    ex = _EXEC_CACHE.get(key)
    if ex is None:
        ex = _Executor(_build_program(meta))
        _EXEC_CACHE[key] = ex
    return ex


class _Result:
    def __init__(self, results, exec_time_ns=None):
        self.results = results
        self.exec_time_ns = exec_time_ns


def _run(inputs, meta, cores, denom_inv=None, profile=False):
    import time

    ex = _get_exec(meta)
    in_maps = _make_in_maps(inputs, meta, cores)
    res = _Result(ex.run(in_maps))
    if profile:
        # no NTFF hook in this container: report min warm wall-clock of a
        # full dispatch (host input concat + tunnel transfer + device exec
        # + output fetch; upper bound on device time). The axon relay has
        # tens-of-ms jitter, so take the min over a larger sample.
        best = None
        for _ in range(20):
            t0 = time.time()
            res.results = ex.run(in_maps)
            dt = time.time() - t0
            best = dt if best is None else min(best, dt)
        res.exec_time_ns = int(best * 1e9)
    raw = np.concatenate([res.results[c]["out"] for c in range(NCORES)],
                         axis=0)
    q = raw[:, :O].astype(np.float32)
    sc = np.ascontiguousarray(raw[:, O:O + 2]).view(np.float16)
    out = q * sc.astype(np.float32)
    return out, res


def kernel(x, edge_index, edge_type, feat2c_W, feat2c_b, rgcn_W, rgcn_root,
           rgcn_bias, ln_gamma, ln_beta):
    inputs = dict(x=x, edge_index=edge_index, edge_type=edge_type,
                  feat2c_W=feat2c_W, feat2c_b=feat2c_b, rgcn_W=rgcn_W,
                  rgcn_root=rgcn_root, rgcn_bias=rgcn_bias,
                  ln_gamma=ln_gamma, ln_beta=ln_beta)
    meta, cores, _ = _preprocess(
        np.asarray(x), np.asarray(edge_index), np.asarray(edge_type))
    out, _ = _run(inputs, meta, cores, profile=False)
    return out


if __name__ == "__main__":
    pass

